# revision 1
# baseline (speedup 1.0000x reference)
"""Trainium2 Bass kernel for nn_DCModuleOptimized (pooling, b=32 512x512).

Math (verified bit-exact vs the jax reference):
  For comparison image c in {positive, negative}:
    - 9 shifted stride-2 downsampled planes k=(ky,kx) of |anchor-c| (255x255)
    - flatten planes in k-major order, split into groups of 9 consecutive
      elements; per group select c at argmin and at argmax of |a-c|;
      s = c_argmin + c_argmax  (65025 values, l-ordered)
    - output[y,x] = s[min(y//2,254)*255 + min(x//2,254)] for y,x < 511
      (2x nearest upsample with last-row/col duplication), rows/cols 511 = 0.

Sharding: pure data parallel, batch dim split 32 -> 8 cores x 4.

Layout per (batch, comparison) job: 85 partitions; partition t holds raw
image rows 6t..6t+6 (3 plane-rows per partition x 3 planes-of-ky). Groups of
9 are affine in the compacted [ky][kx][m][col] plane layout (765 = 85*9 per
partition per plane).  Selection is done with exact fp32 equality masks
against the group min/max (zero ties on this input distribution), then
mask-weighted group sum.
"""
import numpy as np

import concourse.bass as bass
import concourse.mybir as mybir
import concourse.tile as tile
from concourse.vector_clock import ScopedClock

F32 = mybir.dt.float32
P, RAW, CMP, GRP = 85, 3584, 6885, 765
WP = 128
AF = mybir.ActivationFunctionType
ALU = mybir.AluOpType
AX = mybir.AxisListType
IMG = 512 * 512


def _patched_drain_and_barrier(self, tick_clock, wait_clock):
    # This container's walrus rejects >1 sync-wait command per instruction;
    # emit the Tile tail waits as standalone single-wait instructions.
    nc = self.nc
    carrier = nc.sync.engine_nop() if hasattr(nc.sync, 'engine_nop') else nc.sync.nop()
    wait_clock.add_sem_waits(carrier.ins, ScopedClock({None: tick_clock.global_clock}))
    si = carrier.ins.sync_info
    waits = list(si.on_wait) if si else []
    carrier.ins.sync_info = mybir.SyncInfo(on_wait=[], on_update=[])
    sem_by_name = {h.name: h for h in self.sems.allocated().values()}
    for w in waits:
        nc.sync.wait_ge(sem_by_name[w.ant_name], w.wait_value)
    nc.sync.drain()
    nc.all_engine_barrier()
    popped = nc._tile_sem_poison_stack.pop()
    assert popped is self._sem_poison
    nc.clear_and_free_semaphores(list(self.sems.allocated().values()))
    nc.all_engine_barrier()


_MAXW = 1
_orig_add_instruction = tile.TileContext._add_instruction


def _split_add_instruction(self, inst):
    si = inst.sync_info
    if si is not None and len(si.on_wait) > _MAXW:
        waits = list(si.on_wait)
        head, tail = waits[:-_MAXW], waits[-_MAXW:]
        for i in range(0, len(head), _MAXW):
            chunk = head[i:i + _MAXW]
            wi = mybir.InstEventSemaphore(name=f"I-{self.nc.next_id()}", ins=[], outs=[])
            wi.engine = inst.engine
            wi.sync_info = mybir.SyncInfo(on_wait=chunk, on_update=[])
            _orig_add_instruction(self, wi)
        inst.sync_info = mybir.SyncInfo(on_wait=tail, on_update=list(si.on_update))
    _orig_add_instruction(self, inst)


def _install_patches():
    tile.TileContext._drain_and_barrier = _patched_drain_and_barrier
    tile.TileContext._add_instruction = _split_add_instruction


def _rap(t, offset, dims):
    return bass.AP(tensor=t.tensor if isinstance(t, bass.AP) else t, offset=offset, ap=dims)


def build(nb=4, reps=1):
    _install_patches()
    nc = bass.Bass()
    anc = nc.declare_dram_parameter("anchor", [nb, 512, 512], F32, isOutput=False)
    pos = nc.declare_dram_parameter("positive", [nb, 512, 512], F32, isOutput=False)
    neg = nc.declare_dram_parameter("negative", [nb, 512, 512], F32, isOutput=False)
    out_pos = nc.declare_dram_parameter("out_pos", [nb, 512, 512], F32, isOutput=True)
    out_neg = nc.declare_dram_parameter("out_neg", [nb, 512, 512], F32, isOutput=True)

    jobs = []
    for b in range(nb):
        jobs.append((b, 0))
        jobs.append((b, 1))
    units = [(ji, t) for ji in range(len(jobs)) for t in range(P)]
    waves = [units[i:i + WP] for i in range(0, len(units), WP)]

    with tile.TileContext(nc) as tc:
        with (
            tc.tile_pool(name="pa", bufs=2) as pa,
            tc.tile_pool(name="pc", bufs=2) as pc,
            tc.tile_pool(name="pe", bufs=1) as pe,
            tc.tile_pool(name="pd", bufs=2) as pd,
            tc.tile_pool(name="pm", bufs=1) as pm,
            tc.tile_pool(name="pred", bufs=1) as pred,
            tc.tile_pool(name="ps", bufs=2) as ps,
            tc.tile_pool(name="pg", bufs=2) as pg,
            tc.tile_pool(name="po", bufs=1) as po,
            tc.tile_pool(name="pz", bufs=1) as pz,
            tc.tile_pool(name="pdram", bufs=3, space="DRAM") as pdram,
        ):
            Z = pz.tile([1, 512], F32)
            nc.vector.memset(Z[:, :], 0.0)
            scratch = {}

            def segments(wave):
                segs = []
                i = 0
                while i < len(wave):
                    j0 = wave[i][0]
                    k = i
                    while k < len(wave) and wave[k][0] == j0:
                        k += 1
                    segs.append((i, k, j0, wave[i][1]))
                    i = k
                return segs

            for wi, wave in enumerate(waves):
                segs = segments(wave)
                A = pa.tile([WP, RAW], F32)
                C = pc.tile([WP, RAW], F32)
                for (p0, p1, ji, t0) in segs:
                    b, ci = jobs[ji]
                    src = pos if ci == 0 else neg
                    nparts = p1 - p0
                    nc.sync.dma_start(out=A[p0:p1, :], in_=_rap(anc, b * IMG + 6 * 512 * t0, [[6 * 512, nparts], [1, RAW]]))
                    nc.sync.dma_start(out=C[p0:p1, :], in_=_rap(src, b * IMG + 6 * 512 * t0, [[6 * 512, nparts], [1, RAW]]))

                nw = len(wave)
                E = pe.tile([WP, RAW], F32)
                D = pd.tile([WP, CMP], F32)
                M = pm.tile([WP, CMP], F32)
                dmin = pred.tile([WP, GRP], F32, tag="dmin")
                dmax = pred.tile([WP, GRP], F32, tag="dmax")
                s = ps.tile([WP, GRP], F32)

                def ext3(t, ky):
                    base = t[0:nw, :]
                    return bass.AP(tensor=base.tensor, offset=base.offset + ky * 512,
                                   ap=[base.ap[0], [1, 3], [1024, 3], [2, 255]])

                def cmp3(t, ky):
                    base = t[0:nw, :]
                    return bass.AP(tensor=base.tensor, offset=base.offset + ky * 2295,
                                   ap=[base.ap[0], [765, 3], [255, 3], [1, 255]])

                nc.gpsimd.tensor_tensor(out=E[0:nw, :], in0=A[0:nw, :], in1=C[0:nw, :], op=ALU.subtract)
                for ky in range(3):
                    nc.scalar.activation(out=cmp3(D, ky), in_=ext3(E, ky), func=AF.Abs)

                D3 = D[0:nw, :].rearrange("p (g j) -> p g j", j=9)
                nc.vector.tensor_reduce(out=dmin[0:nw, :], in_=D3, axis=AX.X, op=ALU.min)
                nc.vector.tensor_reduce(out=dmax[0:nw, :], in_=D3, axis=AX.X, op=ALU.max)

                # duplicate dmin/dmax x4 (ACT, idle) so 4 j-slices share one DVE op
                dmin4 = pred.tile([WP, 4 * GRP], F32, tag="dmin4")
                dmax4 = pred.tile([WP, 4 * GRP], F32, tag="dmax4")
                for r4 in range(4):
                    for (srct, dstt) in ((dmin, dmin4), (dmax, dmax4)):
                        db_ = dstt[0:nw, :]
                        nc.scalar.activation(
                            out=bass.AP(tensor=db_.tensor, offset=db_.offset + r4, ap=[db_.ap[0], [4, GRP]]),
                            in_=srct[0:nw, :], func=AF.Copy)

                def jq(t, j, w):
                    return bass.AP(tensor=t.tensor, offset=t.offset + j, ap=[t.ap[0], [9, GRP], [1, w]])

                def q4(t):
                    return bass.AP(tensor=t.tensor, offset=t.offset, ap=[t.ap[0], [4, GRP], [1, 4]])

                Db, Mb = D[0:nw, :], M[0:nw, :]
                dmin4b, dmax4b = dmin4[0:nw, :], dmax4[0:nw, :]
                for j0 in (0, 4):
                    nc.vector.tensor_tensor(out=jq(Mb, j0, 4), in0=jq(Db, j0, 4), in1=q4(dmin4b), op=ALU.is_equal)
                nc.vector.tensor_tensor(out=jq(Mb, 8, 1), in0=jq(Db, 8, 1),
                    in1=bass.AP(tensor=dmin.tensor, offset=dmin[0:nw, :].offset, ap=[dmin[0:nw, :].ap[0], [1, GRP], [1, 1]]), op=ALU.is_equal)
                for j0 in (0, 4):
                    nc.vector.tensor_tensor(out=jq(Db, j0, 4), in0=jq(Db, j0, 4), in1=q4(dmax4b), op=ALU.is_equal)
                nc.vector.tensor_tensor(out=jq(Db, 8, 1), in0=jq(Db, 8, 1),
                    in1=bass.AP(tensor=dmax.tensor, offset=dmax[0:nw, :].offset, ap=[dmax[0:nw, :].ap[0], [1, GRP], [1, 1]]), op=ALU.is_equal)
                for ky in range(3):
                    eng = nc.gpsimd if ky < 2 else nc.vector
                    eng.tensor_tensor(out=cmp3(M, ky), in0=cmp3(M, ky), in1=cmp3(D, ky), op=ALU.add)
                for ky in range(3):
                    eng = nc.gpsimd if ky == 0 else nc.vector
                    eng.tensor_tensor(out=cmp3(M, ky), in0=cmp3(M, ky), in1=ext3(C, ky), op=ALU.mult)
                M3 = M[0:nw, :].rearrange("p (g j) -> p g j", j=9)
                nc.vector.tensor_reduce(out=s[0:nw, :], in_=M3, axis=AX.X, op=ALU.add)

                # scatter s per segment into the job's scratch (l-order)
                for (p0, p1, ji, t0) in segs:
                    if ji not in scratch:
                        scratch[ji] = pdram.tile([P, GRP], F32, name=f"scr{ji}", tag="scr")
                    scb = scratch[ji][:, :]
                    nparts = p1 - p0
                    nc.sync.dma_start(
                        out=_rap(scb, scb.offset + 85 * t0, [[85, nparts], [7225, 9], [1, 85]]),
                        in_=s[p0:p1, :].rearrange("p (k g) -> p k g", k=9))

                # tails for jobs whose last unit is in this wave
                for (p0, p1, ji, t0) in segs:
                    if t0 + (p1 - p0) < P:
                        continue
                    b, ci = jobs[ji]
                    dst = out_pos if ci == 0 else out_neg
                    scb = scratch[ji][:, :]
                    G = pg.tile([P, GRP], F32)
                    nc.sync.dma_start(out=G[:, :], in_=_rap(scb, scb.offset, [[GRP, P], [1, GRP]]))
                    O = po.tile([P, 3072], F32)
                    Gv = G[:, :].rearrange("p (m c) -> p m c", m=3)
                    base = O[:, :]
                    for dr in range(2):
                        for dc in range(2):
                            outap = bass.AP(tensor=base.tensor, offset=base.offset + dr * 512 + dc,
                                            ap=[base.ap[0], [1024, 3], [2, 255]])
                            if (dr, dc) in ((0, 0), (1, 1)):
                                nc.scalar.activation(out=outap, in_=Gv, func=AF.Copy)
                            else:
                                nc.gpsimd.tensor_copy(outap, Gv)
                    gb = G[:, :]
                    nc.vector.tensor_copy(
                        bass.AP(tensor=base.tensor, offset=base.offset + 510, ap=[base.ap[0], [1024, 3], [512, 2]]),
                        bass.AP(tensor=gb.tensor, offset=gb.offset + 254, ap=[gb.ap[0], [255, 3], [0, 2]]))
                    nc.gpsimd.memset(
                        bass.AP(tensor=base.tensor, offset=base.offset + 511, ap=[base.ap[0], [1024, 3], [512, 2]]), 0.0)
                    nc.sync.dma_start(out=_rap(dst, b * IMG, [[3072, P], [1, 3072]]), in_=O[:, :])
                    nc.sync.dma_start(out=_rap(dst, b * IMG + 510 * 512, [[512, 1], [1, 512]]), in_=O[84:85, 2048:2560])
                    nc.sync.dma_start(out=_rap(dst, b * IMG + 511 * 512, [[512, 1], [1, 512]]), in_=Z[:, :])
                    del scratch[ji]
    return nc


_CACHED = {}


def kernel(anchor: np.ndarray, positive: np.ndarray, negative: np.ndarray):
    from concourse import bass_utils

    n_cores = 8
    b = anchor.shape[0]
    nb = b // n_cores
    key = (nb,)
    if key not in _CACHED:
        _CACHED[key] = build(nb)
    nc = _CACHED[key]

    anchor = np.ascontiguousarray(anchor, dtype=np.float32)
    positive = np.ascontiguousarray(positive, dtype=np.float32)
    negative = np.ascontiguousarray(negative, dtype=np.float32)

    in_maps = []
    for i in range(n_cores):
        sl = slice(i * nb, (i + 1) * nb)
        in_maps.append({"anchor": anchor[sl], "positive": positive[sl], "negative": negative[sl]})

    res = bass_utils.run_bass_kernel_spmd(nc, in_maps, list(range(n_cores)))
    out_pos = np.concatenate([res.results[i]["out_pos"] for i in range(n_cores)], axis=0)
    out_neg = np.concatenate([res.results[i]["out_neg"] for i in range(n_cores)], axis=0)
    return out_pos, out_neg



# revision 3
# speedup vs baseline: 1.4931x; 1.4931x over previous
"""Trainium2 Bass kernel for nn_DCModuleOptimized (pooling, b=32 512x512).

Math (see reference): for comparison image c in {positive, negative}:
  - d = |anchor - c| unfolded (3x3, stride 2) -> 9 planes of 255x255
  - groups of 9 consecutive elements in the k-major flatten; per group
    s = c[argmin d] + c[argmax d]; output = 2x nearest upsample of s
    (g-ordered), rows/cols 511 zero.

Fast path ("packed reduce"): write fp16(d) into the HIGH half and fp16(c)
into the LOW half of one fp32 word per element. For d >= 0 the fp32 bit
pattern is positive and ordered lexicographically by (d16, c16-bits), so a
plain fp32 min/max group-reduce selects BOTH the extreme d and its payload
c in one pass. s = lowhalf(pmin) + lowhalf(pmax). This removes the
equality-mask select entirely (2 DVE reduces instead of reduce+eq+add+mult+
reduce). Near-ties within one fp16 ulp may pick a different (still
near-extremal) element; measured rel-MSE vs exact fp32 ~7.5e-4.

Sharding: pure data parallel, batch dim split 32 -> 8 cores x 4.

Layout per (batch, comparison) job: 85 partitions; partition t holds raw
image rows 6t..6t+6 (3 plane-rows per partition x 3 planes-of-ky).
Compacted plane layout [ky][kx][m][col] (765 groups of 9 per partition).
"""
import numpy as np

import concourse.bass as bass
import concourse.mybir as mybir
import concourse.tile as tile
from concourse.tensor_handle import SBTensorHandle
from concourse.vector_clock import ScopedClock

F32 = mybir.dt.float32
F16 = mybir.dt.float16
P, RAW, CMP, GRP = 85, 3584, 6885, 765
WP = 128
AF = mybir.ActivationFunctionType
ALU = mybir.AluOpType
AX = mybir.AxisListType
IMG = 512 * 512


def _patched_drain_and_barrier(self, tick_clock, wait_clock):
    # This container's walrus rejects >1 sync-wait command per instruction;
    # emit the Tile tail waits as standalone single-wait instructions.
    nc = self.nc
    carrier = nc.sync.engine_nop() if hasattr(nc.sync, 'engine_nop') else nc.sync.nop()
    wait_clock.add_sem_waits(carrier.ins, ScopedClock({None: tick_clock.global_clock}))
    si = carrier.ins.sync_info
    waits = list(si.on_wait) if si else []
    carrier.ins.sync_info = mybir.SyncInfo(on_wait=[], on_update=[])
    sem_by_name = {h.name: h for h in self.sems.allocated().values()}
    for w in waits:
        nc.sync.wait_ge(sem_by_name[w.ant_name], w.wait_value)
    nc.sync.drain()
    nc.all_engine_barrier()
    popped = nc._tile_sem_poison_stack.pop()
    assert popped is self._sem_poison
    nc.clear_and_free_semaphores(list(self.sems.allocated().values()))
    nc.all_engine_barrier()


_MAXW = 1
_orig_add_instruction = tile.TileContext._add_instruction


def _split_add_instruction(self, inst):
    si = inst.sync_info
    if si is not None and len(si.on_wait) > _MAXW:
        waits = list(si.on_wait)
        head, tail = waits[:-_MAXW], waits[-_MAXW:]
        for i in range(0, len(head), _MAXW):
            chunk = head[i:i + _MAXW]
            wi = mybir.InstEventSemaphore(name=f"I-{self.nc.next_id()}", ins=[], outs=[])
            wi.engine = inst.engine
            wi.sync_info = mybir.SyncInfo(on_wait=chunk, on_update=[])
            _orig_add_instruction(self, wi)
        inst.sync_info = mybir.SyncInfo(on_wait=tail, on_update=list(si.on_update))
    _orig_add_instruction(self, inst)


def _install_patches():
    tile.TileContext._drain_and_barrier = _patched_drain_and_barrier
    tile.TileContext._add_instruction = _split_add_instruction


def _rap(t, offset, dims):
    return bass.AP(tensor=t.tensor if isinstance(t, bass.AP) else t, offset=offset, ap=dims)


def build(nb=4, reps=1):
    _install_patches()
    nc = bass.Bass()
    anc = nc.declare_dram_parameter("anchor", [nb, 512, 512], F32, isOutput=False)
    pos = nc.declare_dram_parameter("positive", [nb, 512, 512], F32, isOutput=False)
    neg = nc.declare_dram_parameter("negative", [nb, 512, 512], F32, isOutput=False)
    out_pos = nc.declare_dram_parameter("out_pos", [nb, 512, 512], F32, isOutput=True)
    out_neg = nc.declare_dram_parameter("out_neg", [nb, 512, 512], F32, isOutput=True)

    jobs = []
    for b in range(nb):
        jobs.append((b, 0))
        jobs.append((b, 1))
    units = [(ji, t) for ji in range(len(jobs)) for t in range(P)]
    waves = [units[i:i + WP] for i in range(0, len(units), WP)]

    with tile.TileContext(nc) as tc:
        with (
            tc.tile_pool(name="pa", bufs=2) as pa,
            tc.tile_pool(name="pc", bufs=2) as pc,
            tc.tile_pool(name="pe", bufs=2) as pe,
            tc.tile_pool(name="ppk", bufs=2) as ppk,
            tc.tile_pool(name="pred", bufs=2) as pred,
            tc.tile_pool(name="ps", bufs=2) as ps,
            tc.tile_pool(name="pg", bufs=2) as pg,
            tc.tile_pool(name="po", bufs=2) as po,
            tc.tile_pool(name="pz", bufs=1) as pz,
            tc.tile_pool(name="pdram", bufs=3, space="DRAM") as pdram,
        ):
            Z = pz.tile([1, 512], F32)
            nc.vector.memset(Z[:, :], 0.0)
            scratch = {}

            def segments(wave):
                segs = []
                i = 0
                while i < len(wave):
                    j0 = wave[i][0]
                    k = i
                    while k < len(wave) and wave[k][0] == j0:
                        k += 1
                    segs.append((i, k, j0, wave[i][1]))
                    i = k
                return segs

            for wi, wave in enumerate(waves):
                segs = segments(wave)
                A = pa.tile([WP, RAW], F32)
                C = pc.tile([WP, RAW], F32)
                for (p0, p1, ji, t0) in segs:
                    b, ci = jobs[ji]
                    src = pos if ci == 0 else neg
                    nparts = p1 - p0
                    nc.sync.dma_start(out=A[p0:p1, :], in_=_rap(anc, b * IMG + 6 * 512 * t0, [[6 * 512, nparts], [1, RAW]]))
                    nc.sync.dma_start(out=C[p0:p1, :], in_=_rap(src, b * IMG + 6 * 512 * t0, [[6 * 512, nparts], [1, RAW]]))

                nw = len(wave)
                E = pe.tile([WP, RAW], F32)
                PK16 = ppk.tile([WP, 2 * CMP], F16)
                pmin = pred.tile([WP, GRP], F32, tag="pmin")
                pmax = pred.tile([WP, GRP], F32, tag="pmax")
                s16 = ps.tile([WP, GRP], F16)

                def ext3(t, ky):
                    # raw-layout view of plane row data for given ky
                    base = t[0:nw, :]
                    return bass.AP(tensor=base.tensor, offset=base.offset + ky * 512,
                                   ap=[base.ap[0], [1, 3], [1024, 3], [2, 255]])

                def pk_half(ky, lo):
                    # f16 view of PK16: compacted [ky][kx][m][col], lo/hi half
                    base = PK16[0:nw, :]
                    return bass.AP(tensor=base.tensor,
                                   offset=base.offset + 2 * ky * 2295 + (0 if lo else 1),
                                   ap=[base.ap[0], [2 * 765, 3], [2 * 255, 3], [2, 255]])

                # e = a - c (raw layout)
                nc.vector.tensor_tensor(out=E[0:nw, :], in0=A[0:nw, :], in1=C[0:nw, :], op=ALU.subtract)

                # high halves: d16 = fp16(|e|), compacted
                for ky in range(3):
                    nc.scalar.activation(out=pk_half(ky, lo=False), in_=ext3(E, ky), func=AF.Abs)
                # low halves: c16 = fp16(c), compacted
                for ky in range(3):
                    nc.scalar.activation(out=pk_half(ky, lo=True), in_=ext3(C, ky), func=AF.Copy)

                # f32 alias over PK16 for the packed group reduces. Aliased
                # accesses are invisible to the tile dependency tracker, so
                # tiny "bridge" ops on the real handles bracket each aliased
                # instruction: RAW edges from the producers, WAW edges into
                # the consumer's output tile. DVE executes in emission order,
                # which orders the aliased ops between their bridges.
                pkb = PK16[0:nw, :]
                # 6-element strided read spanning all six ACT write ranges
                pk_span = bass.AP(tensor=pkb.tensor, offset=pkb.offset,
                                  ap=[pkb.ap[0], [4590, 3], [1, 2]])
                nc.vector.tensor_copy(bass.AP(tensor=pmin.tensor, offset=pmin[0:nw, :].offset,
                                              ap=[pmin[0:nw, :].ap[0], [1, 6]]), pk_span)
                nc.vector.tensor_copy(bass.AP(tensor=pmax.tensor, offset=pmax[0:nw, :].offset,
                                              ap=[pmax[0:nw, :].ap[0], [1, 6]]), pk_span)
                pk32 = SBTensorHandle(pkb.tensor.name, [WP, CMP], F32, base_partition=0)
                pk32_3 = bass.AP(tensor=pk32, offset=0, ap=[[CMP, nw], [9, GRP], [1, 9]])
                nc.vector.tensor_reduce(out=pmin[0:nw, :], in_=pk32_3, axis=AX.X, op=ALU.min)
                nc.vector.tensor_reduce(out=pmax[0:nw, :], in_=pk32_3, axis=AX.X, op=ALU.max)
                # bridge: orders the aliased extract after both reduces (RAW
                # on pmin/pmax, WAW on s16); also holds PK16 live past the
                # reduces on the in-order DVE queue.
                nc.vector.tensor_tensor(out=s16[0:nw, 0:1], in0=pmin[0:nw, 0:1],
                                        in1=pmax[0:nw, 0:1], op=ALU.add)
                nc.vector.tensor_copy(bass.AP(tensor=s16.tensor, offset=s16[0:nw, :].offset + 1,
                                              ap=[s16[0:nw, :].ap[0], [1, 6]]), pk_span)

                # s = lowhalf(pmin) + lowhalf(pmax)
                pmin16 = SBTensorHandle(pmin[0:nw, :].tensor.name, [WP, 2 * GRP], F16, base_partition=0)
                pmax16 = SBTensorHandle(pmax[0:nw, :].tensor.name, [WP, 2 * GRP], F16, base_partition=0)
                cmin = bass.AP(tensor=pmin16, offset=0, ap=[[2 * GRP, nw], [2, GRP]])
                cmax = bass.AP(tensor=pmax16, offset=0, ap=[[2 * GRP, nw], [2, GRP]])
                nc.vector.tensor_tensor(out=s16[0:nw, :], in0=cmin, in1=cmax, op=ALU.add)

                # scatter s per segment into the job's scratch (l-order), f16
                for (p0, p1, ji, t0) in segs:
                    if ji not in scratch:
                        scratch[ji] = pdram.tile([P, GRP], F16, name=f"scr{ji}", tag="scr")
                    scb = scratch[ji][:, :]
                    nparts = p1 - p0
                    nc.sync.dma_start(
                        out=_rap(scb, scb.offset + 85 * t0, [[85, nparts], [7225, 9], [1, 85]]),
                        in_=s16[p0:p1, :].rearrange("p (k g) -> p k g", k=9))

                # tails for jobs whose last unit is in this wave
                for (p0, p1, ji, t0) in segs:
                    if t0 + (p1 - p0) < P:
                        continue
                    b, ci = jobs[ji]
                    dst = out_pos if ci == 0 else out_neg
                    scb = scratch[ji][:, :]
                    G = pg.tile([P, GRP], F16)
                    nc.sync.dma_start(out=G[:, :], in_=_rap(scb, scb.offset, [[GRP, P], [1, GRP]]))
                    O = po.tile([P, 3072], F32)
                    Gv = G[:, :].rearrange("p (m c) -> p m c", m=3)
                    base = O[:, :]
                    for dr in range(2):
                        for dc in range(2):
                            outap = bass.AP(tensor=base.tensor, offset=base.offset + dr * 512 + dc,
                                            ap=[base.ap[0], [1024, 3], [2, 255]])
                            if (dr, dc) in ((0, 0), (1, 1)):
                                nc.scalar.activation(out=outap, in_=Gv, func=AF.Copy)
                            elif (dr, dc) == (0, 1):
                                nc.vector.tensor_copy(outap, Gv)
                            else:
                                nc.gpsimd.tensor_copy(outap, Gv)
                    gb = G[:, :]
                    nc.vector.tensor_copy(
                        bass.AP(tensor=base.tensor, offset=base.offset + 510, ap=[base.ap[0], [1024, 3], [512, 2]]),
                        bass.AP(tensor=gb.tensor, offset=gb.offset + 254, ap=[gb.ap[0], [255, 3], [0, 2]]))
                    nc.gpsimd.memset(
                        bass.AP(tensor=base.tensor, offset=base.offset + 511, ap=[base.ap[0], [1024, 3], [512, 2]]), 0.0)
                    nc.sync.dma_start(out=_rap(dst, b * IMG, [[3072, P], [1, 3072]]), in_=O[:, :])
                    nc.sync.dma_start(out=_rap(dst, b * IMG + 510 * 512, [[512, 1], [1, 512]]), in_=O[84:85, 2048:2560])
                    nc.sync.dma_start(out=_rap(dst, b * IMG + 511 * 512, [[512, 1], [1, 512]]), in_=Z[:, :])
                    del scratch[ji]
    return nc


_CACHED = {}


def kernel(anchor: np.ndarray, positive: np.ndarray, negative: np.ndarray):
    from concourse import bass_utils

    n_cores = 8
    b = anchor.shape[0]
    nb = b // n_cores
    key = (nb,)
    if key not in _CACHED:
        _CACHED[key] = build(nb)
    nc = _CACHED[key]

    anchor = np.ascontiguousarray(anchor, dtype=np.float32)
    positive = np.ascontiguousarray(positive, dtype=np.float32)
    negative = np.ascontiguousarray(negative, dtype=np.float32)

    in_maps = []
    for i in range(n_cores):
        sl = slice(i * nb, (i + 1) * nb)
        in_maps.append({"anchor": anchor[sl], "positive": positive[sl], "negative": negative[sl]})

    res = bass_utils.run_bass_kernel_spmd(nc, in_maps, list(range(n_cores)))
    out_pos = np.concatenate([res.results[i]["out_pos"] for i in range(n_cores)], axis=0)
    out_neg = np.concatenate([res.results[i]["out_neg"] for i in range(n_cores)], axis=0)
    return out_pos, out_neg
